# revision 8
# baseline (speedup 1.0000x reference)
"""MoE layer (E=16 experts, top-2 routing) on 8 Trainium2 NeuronCores.

Strategy (expert-parallel, all-to-all dispatch done host-side):
  1. Host computes the gate (logits -> softmax -> top-2 -> renormalized
     combine weights) in fp32 numpy, exactly mirroring the reference ops.
  2. Tokens are gathered per expert; experts are bin-packed onto the 8
     cores (for the default load distribution this is exactly one
     5-block expert + one 4-block expert per core, no expert splitting).
  3. Each core runs the two expert FFNs on its gathered tokens with a
     Bass/Tile kernel: all activations live transposed ([feature, token])
     so both GEMMs keep weights stationary in the PE array and no
     on-chip transposes are needed.  bf16 inputs, fp32 PSUM accumulation,
     erf-Gelu on the scalar engine, combine-weight scaling on the vector
     engine.
  4. Host scatter-adds the two weighted contributions per token back to
     the [B, S, D] output.

Self-contained: hardcodes the problem shapes; no sibling imports.
"""

import numpy as np
import ml_dtypes

import concourse.bass as bass
import concourse.bacc as bacc
import concourse.mybir as mybir
import concourse.tile as tile
from concourse.bass_utils import run_bass_kernel_spmd

BF16 = ml_dtypes.bfloat16
F32 = np.float32

D = 512          # model dim
F = 2048         # FFN hidden dim
E = 16           # experts
TOPK = 2
N_CORES = 8
BLK = 512        # token block streamed per matmul (moving operand width)
KD = D // 128    # 4  contraction k-tiles for GEMM1
KF = F // 128    # 16 contraction k-tiles for GEMM2
MD = D // 128    # 4  output m-tiles for GEMM2

# Set by test harness only (profiling).  The graded path leaves these alone.
TRACE = False
TRACE_KW = {}
LAST_RESULT = None


def _gate(xf, gate_w):
    """fp32 gate identical to the reference: softmax -> top-2 -> renorm."""
    logits = xf @ gate_w                       # [T, E] fp32
    m = logits.max(axis=1, keepdims=True)
    ex = np.exp(logits - m)
    rw = ex / ex.sum(axis=1, keepdims=True)
    order = np.argsort(-rw, axis=1, kind="stable")
    sel = order[:, :TOPK]
    topw = np.take_along_axis(rw, sel, axis=1)
    topw = topw / topw.sum(axis=1, keepdims=True)
    return sel, topw.astype(F32)


def _plan(items):
    """Bin-pack expert workloads onto N_CORES cores.

    items: list of [expert, tokens, ks, ws] with len(tokens) > 0.
    Returns (bins, caps): bins[c] = list of items (sorted desc by size),
    caps[j] = token capacity (multiple of 128) of slot j, shared by all
    cores so every core compiles to the identical program.
    """
    nblk = lambda n: -(-n // BLK)
    btot = sum(nblk(len(it[1])) for it in items)
    bcap = max(1, -(-btot // N_CORES))
    while True:
        work = []
        for e, toks, ks, ws in items:
            o = 0
            while len(toks) - o > bcap * BLK:
                work.append([e, toks[o:o + bcap * BLK], ks[o:o + bcap * BLK],
                             ws[o:o + bcap * BLK]])
                o += bcap * BLK
            work.append([e, toks[o:], ks[o:], ws[o:]])
        work.sort(key=lambda it: -len(it[1]))
        bins = [[] for _ in range(N_CORES)]
        used = [0] * N_CORES
        ok = True
        for it in work:
            b = nblk(len(it[1]))
            for c in sorted(range(N_CORES), key=lambda c: used[c]):
                if used[c] + b <= bcap:
                    bins[c].append(it)
                    used[c] += b
                    break
            else:
                ok = False
                break
        if ok:
            break
        bcap += 1
    for b in bins:
        b.sort(key=lambda it: -len(it[1]))
    nslot = max(len(b) for b in bins)
    caps = []
    for j in range(nslot):
        cap = max((len(b[j][1]) if j < len(b) else 0) for b in bins)
        caps.append(-(-cap // 128) * 128)
    return bins, caps


_NC_CACHE = {}


def _build(nslot, caps, use_b1, use_b2):
    """Build the per-core Bass program (identical across cores)."""
    key = (nslot, tuple(caps), use_b1, use_b2)
    if key in _NC_CACHE:
        return _NC_CACHE[key]

    ntok = sum(caps)
    nc = bacc.Bacc("TRN2", target_bir_lowering=False, debug=False,
                   num_devices=N_CORES)
    dt = mybir.dt
    xg_d = nc.dram_tensor("xg", [128, KD, ntok], dt.bfloat16,
                          kind="ExternalInput").ap()
    # Weights grouped by consumption order, fully contiguous per partition
    # (4KB DMA lines): w1 group g = F-cols [g*512,(g+1)*512), w2 group c =
    # F-rows [c*512,(c+1)*512).
    w1_d = nc.dram_tensor("w1g", [nslot, F // BLK, 128, KD, BLK], dt.bfloat16,
                          kind="ExternalInput").ap()
    w2_d = nc.dram_tensor("w2g", [nslot, F // BLK, 128, BLK // 128, D],
                          dt.bfloat16, kind="ExternalInput").ap()
    cw_d = nc.dram_tensor("cwg", [128, ntok], dt.float32,
                          kind="ExternalInput").ap()
    b1_d = (nc.dram_tensor("b1g", [nslot, 128, KF], dt.float32,
                           kind="ExternalInput").ap() if use_b1 else None)
    b2_d = (nc.dram_tensor("b2g", [nslot, 128, MD], dt.float32,
                           kind="ExternalInput").ap() if use_b2 else None)
    yg_d = nc.dram_tensor("ygT", [D, ntok], dt.float32,
                          kind="ExternalOutput").ap()

    gelu = mybir.ActivationFunctionType.Gelu
    ident = mybir.ActivationFunctionType.Identity

    with tile.TileContext(nc) as tc:
        with (
            tc.tile_pool(name="wpool", bufs=2) as wpool,
            tc.tile_pool(name="io", bufs=3) as io,
            tc.tile_pool(name="hpool", bufs=8) as hpool,
            tc.tile_pool(name="ypool", bufs=8) as ypool,
            tc.tile_pool(name="psh", bufs=4, space="PSUM") as psh,
            tc.tile_pool(name="psy", bufs=4, space="PSUM") as psy,
        ):
            col0 = 0
            for s in range(nslot):
                # Weights ride the scalar engine's HWDGE queue so they don't
                # head-of-line-block the activations on the sync queue.
                # Chunked in consumption order: GEMM1 chunk c needs w1
                # group c first, then GEMM2 chunk c needs w2 group c.
                w1ts, w2ts = [], []
                for g in range(F // BLK):
                    w1tg = wpool.tile([128, KD, BLK], dt.bfloat16,
                                      name=f"w1t{s}_{g}", tag=f"w1_{g}")
                    nc.scalar.dma_start(w1tg[:], w1_d[s, g])
                    w1ts.append(w1tg)
                    w2tg = wpool.tile([128, BLK // 128, D], dt.bfloat16,
                                      name=f"w2t{s}_{g}", tag=f"w2_{g}")
                    nc.scalar.dma_start(w2tg[:], w2_d[s, g])
                    w2ts.append(w2tg)
                if use_b1:
                    b1t = wpool.tile([128, KF], dt.float32,
                                     name=f"b1t{s}", tag="b1")
                    nc.scalar.dma_start(b1t[:], b1_d[s])
                if use_b2:
                    b2t = wpool.tile([128, MD], dt.float32,
                                     name=f"b2t{s}", tag="b2")
                    nc.scalar.dma_start(b2t[:], b2_d[s])

                cap = caps[s]
                off = 0
                while off < cap:
                    w = min(BLK, cap - off)
                    c0 = col0 + off
                    xt = io.tile([128, KD, w], dt.bfloat16,
                                 name=f"xt_{s}_{off}", tag="xt")
                    nc.sync.dma_start(xt[:], xg_d[:, :, c0:c0 + w])
                    cwt = io.tile([128, w], dt.float32,
                                  name=f"cw_{s}_{off}", tag="cw")
                    nc.sync.dma_start(cwt[:], cw_d[:, c0:c0 + w])

                    yts = [psy.tile([128, w], dt.float32,
                                    name=f"psy_{s}_{off}_{m}", tag="psy")
                           for m in range(MD)]
                    n_fc = F // BLK           # 4 chunks of 512 hidden dims
                    fpc = BLK // 128          # 4 f-tiles per chunk
                    for c in range(n_fc):
                        hts = []
                        for f in range(fpc):
                            kk = c * fpc + f
                            ph = psh.tile([128, w], dt.float32,
                                          name=f"psh_{s}_{off}_{kk}", tag="psh")
                            for k in range(KD):
                                nc.tensor.matmul(
                                    ph[:],
                                    w1ts[c][:, k, f * 128:(f + 1) * 128],
                                    xt[:, k, :],
                                    start=(k == 0), stop=(k == KD - 1))
                            ht = hpool.tile([128, w], dt.bfloat16,
                                            name=f"ht_{s}_{off}_{kk}", tag="ht")
                            if use_b1:
                                nc.scalar.activation(ht[:], ph[:], gelu,
                                                     bias=b1t[:, kk:kk + 1])
                            else:
                                nc.scalar.activation(ht[:], ph[:], gelu)
                            hts.append(ht)
                        for m in range(MD):
                            for k in range(fpc):
                                nc.tensor.matmul(
                                    yts[m][:],
                                    w2ts[c][:, k, m * 128:(m + 1) * 128],
                                    hts[k][:],
                                    start=(c == 0 and k == 0),
                                    stop=(c == n_fc - 1 and k == fpc - 1))
                    for m in range(MD):
                        src = yts[m]
                        if use_b2:
                            yb = ypool.tile([128, w], dt.float32,
                                            name=f"yb_{s}_{off}_{m}", tag="yb")
                            nc.scalar.activation(yb[:], yts[m][:], ident,
                                                 bias=b2t[:, m:m + 1])
                            src = yb
                        yf = ypool.tile([128, w], dt.float32,
                                        name=f"yf_{s}_{off}_{m}", tag="yf")
                        nc.vector.tensor_mul(yf[:], src[:], cwt[:])
                        # Outputs on the gpsimd queue: keeps stores from
                        # delaying the next block's x/cw loads.
                        nc.gpsimd.dma_start(
                            yg_d[m * 128:(m + 1) * 128, c0:c0 + w], yf[:])
                    off += w
                col0 += cap
    nc.compile()
    _NC_CACHE[key] = nc
    return nc


def kernel(x, gate_w, w1, b1, w2, b2):
    global LAST_RESULT
    x = np.asarray(x, dtype=F32)
    gate_w = np.asarray(gate_w, dtype=F32)
    w1 = np.asarray(w1, dtype=F32)
    b1 = np.asarray(b1, dtype=F32)
    w2 = np.asarray(w2, dtype=F32)
    b2 = np.asarray(b2, dtype=F32)

    b, s, d = x.shape
    T = b * s
    xf = x.reshape(T, d)

    sel, topw = _gate(xf, gate_w)

    # Per-expert token lists: (expert, token idx, which-of-top2, weight)
    items = []
    for e in range(E):
        t0 = np.nonzero(sel[:, 0] == e)[0]
        t1 = np.nonzero(sel[:, 1] == e)[0]
        toks = np.concatenate([t0, t1])
        if toks.size == 0:
            continue
        ks = np.concatenate([np.zeros_like(t0), np.ones_like(t1)])
        ws = np.concatenate([topw[t0, 0], topw[t1, 1]]).astype(F32)
        items.append([e, toks, ks, ws])

    bins, caps = _plan(items)
    nslot = len(caps)
    ntok = sum(caps)
    col_of = np.concatenate([[0], np.cumsum(caps)])

    use_b1 = bool(np.any(b1))
    use_b2 = bool(np.any(b2))

    # Pack per-core device inputs.
    pos = np.zeros((T, TOPK), dtype=np.int64)
    in_maps = []
    for c in range(N_CORES):
        xg = np.zeros((128, KD, ntok), dtype=BF16)
        cw = np.zeros(ntok, dtype=F32)
        ng = F // BLK
        w1h = np.zeros((nslot, ng, 128, KD, BLK), dtype=BF16)
        w2h = np.zeros((nslot, ng, 128, BLK // 128, D), dtype=BF16)
        b1h = np.zeros((nslot, 128, KF), dtype=F32)
        b2h = np.zeros((nslot, 128, MD), dtype=F32)
        for j, (e, toks, ks, ws) in enumerate(bins[c]):
            o = int(col_of[j])
            n = len(toks)
            xg[:, :, o:o + n] = xf[toks].reshape(n, KD, 128).transpose(2, 1, 0)
            cw[o:o + n] = ws
            pos[toks, ks] = c * ntok + o + np.arange(n)
            for g in range(ng):
                w1h[j, g] = (w1[e][:, g * BLK:(g + 1) * BLK]
                             .reshape(KD, 128, BLK).transpose(1, 0, 2))
                w2h[j, g] = (w2[e][g * BLK:(g + 1) * BLK, :]
                             .reshape(BLK // 128, 128, D).transpose(1, 0, 2))
            b1h[j] = b1[e].reshape(KF, 128).T
            b2h[j] = b2[e].reshape(MD, 128).T
        m = {
            "xg": xg,
            "w1g": w1h,
            "w2g": w2h,
            "cwg": np.ascontiguousarray(
                np.broadcast_to(cw[None, :], (128, ntok))),
        }
        if use_b1:
            m["b1g"] = b1h
        if use_b2:
            m["b2g"] = b2h
        in_maps.append(m)

    nc = _build(nslot, caps, use_b1, use_b2)
    res = run_bass_kernel_spmd(nc, in_maps, core_ids=list(range(N_CORES)),
                               trace=TRACE, **TRACE_KW)
    LAST_RESULT = res

    yall = np.concatenate([res.results[c]["ygT"] for c in range(N_CORES)],
                          axis=1)                      # [D, N_CORES*ntok]
    out_t = yall[:, pos[:, 0]] + yall[:, pos[:, 1]]    # [D, T]
    out = np.ascontiguousarray(out_t.T).reshape(b, s, d).astype(F32)
    return out, np.float32(0.0)


# revision 9
# speedup vs baseline: 1.0281x; 1.0281x over previous
"""MoE layer (E=16 experts, top-2 routing) on 8 Trainium2 NeuronCores.

Strategy (expert-parallel, all-to-all dispatch done host-side):
  1. Host computes the gate (logits -> softmax -> top-2 -> renormalized
     combine weights) in fp32 numpy, exactly mirroring the reference ops.
  2. Tokens are gathered per expert; experts are bin-packed onto the 8
     cores (for the default load distribution this is exactly one
     5-block expert + one 4-block expert per core, no expert splitting).
  3. Each core runs the two expert FFNs on its gathered tokens with a
     Bass/Tile kernel: all activations live transposed ([feature, token])
     so both GEMMs keep weights stationary in the PE array and no
     on-chip transposes are needed.  bf16 inputs, fp32 PSUM accumulation,
     erf-Gelu on the scalar engine, combine-weight scaling on the vector
     engine.
  4. Host scatter-adds the two weighted contributions per token back to
     the [B, S, D] output.

Self-contained: hardcodes the problem shapes; no sibling imports.
"""

import numpy as np
import ml_dtypes

import concourse.bass as bass
import concourse.bacc as bacc
import concourse.mybir as mybir
import concourse.tile as tile
from concourse.bass_utils import run_bass_kernel_spmd

BF16 = ml_dtypes.bfloat16
F32 = np.float32

D = 512          # model dim
F = 2048         # FFN hidden dim
E = 16           # experts
TOPK = 2
N_CORES = 8
BLK = 512        # token block streamed per matmul (moving operand width)
KD = D // 128    # 4  contraction k-tiles for GEMM1
KF = F // 128    # 16 contraction k-tiles for GEMM2
MD = D // 128    # 4  output m-tiles for GEMM2

# Set by test harness only (profiling).  The graded path leaves these alone.
TRACE = False
TRACE_KW = {}
LAST_RESULT = None


def _gate(xf, gate_w):
    """fp32 gate identical to the reference: softmax -> top-2 -> renorm."""
    logits = xf @ gate_w                       # [T, E] fp32
    m = logits.max(axis=1, keepdims=True)
    ex = np.exp(logits - m)
    rw = ex / ex.sum(axis=1, keepdims=True)
    order = np.argsort(-rw, axis=1, kind="stable")
    sel = order[:, :TOPK]
    topw = np.take_along_axis(rw, sel, axis=1)
    topw = topw / topw.sum(axis=1, keepdims=True)
    return sel, topw.astype(F32)


def _plan(items):
    """Bin-pack expert workloads onto N_CORES cores.

    items: list of [expert, tokens, ks, ws] with len(tokens) > 0.
    Returns (bins, caps): bins[c] = list of items (sorted desc by size),
    caps[j] = token capacity (multiple of 128) of slot j, shared by all
    cores so every core compiles to the identical program.
    """
    nblk = lambda n: -(-n // BLK)
    btot = sum(nblk(len(it[1])) for it in items)
    bcap = max(1, -(-btot // N_CORES))
    while True:
        work = []
        for e, toks, ks, ws in items:
            o = 0
            while len(toks) - o > bcap * BLK:
                work.append([e, toks[o:o + bcap * BLK], ks[o:o + bcap * BLK],
                             ws[o:o + bcap * BLK]])
                o += bcap * BLK
            work.append([e, toks[o:], ks[o:], ws[o:]])
        work.sort(key=lambda it: -len(it[1]))
        bins = [[] for _ in range(N_CORES)]
        used = [0] * N_CORES
        ok = True
        for it in work:
            b = nblk(len(it[1]))
            for c in sorted(range(N_CORES), key=lambda c: used[c]):
                if used[c] + b <= bcap:
                    bins[c].append(it)
                    used[c] += b
                    break
            else:
                ok = False
                break
        if ok:
            break
        bcap += 1
    for b in bins:
        b.sort(key=lambda it: -len(it[1]))
    nslot = max(len(b) for b in bins)
    caps = []
    for j in range(nslot):
        cap = max((len(b[j][1]) if j < len(b) else 0) for b in bins)
        caps.append(-(-cap // 128) * 128)
    return bins, caps


_NC_CACHE = {}


def _build(nslot, caps, use_b1, use_b2):
    """Build the per-core Bass program (identical across cores)."""
    key = (nslot, tuple(caps), use_b1, use_b2)
    if key in _NC_CACHE:
        return _NC_CACHE[key]

    ntok = sum(caps)
    nc = bacc.Bacc("TRN2", target_bir_lowering=False, debug=False,
                   num_devices=N_CORES)
    dt = mybir.dt
    xg_d = nc.dram_tensor("xg", [128, KD, ntok], dt.bfloat16,
                          kind="ExternalInput").ap()
    # Weights grouped by consumption order, fully contiguous per partition
    # (4KB DMA lines): w1 group g = F-cols [g*512,(g+1)*512), w2 group c =
    # F-rows [c*512,(c+1)*512).
    w1_d = nc.dram_tensor("w1g", [nslot, F // BLK, 128, KD, BLK], dt.bfloat16,
                          kind="ExternalInput").ap()
    w2_d = nc.dram_tensor("w2g", [nslot, F // BLK, 128, BLK // 128, D],
                          dt.bfloat16, kind="ExternalInput").ap()
    cw_d = nc.dram_tensor("cwg", [128, ntok], dt.float32,
                          kind="ExternalInput").ap()
    b1_d = (nc.dram_tensor("b1g", [nslot, 128, KF], dt.float32,
                           kind="ExternalInput").ap() if use_b1 else None)
    b2_d = (nc.dram_tensor("b2g", [nslot, 128, MD], dt.float32,
                           kind="ExternalInput").ap() if use_b2 else None)
    yg_d = nc.dram_tensor("ygT", [D, ntok], dt.float32,
                          kind="ExternalOutput").ap()

    gelu = mybir.ActivationFunctionType.Gelu
    ident = mybir.ActivationFunctionType.Identity

    with tile.TileContext(nc) as tc:
        with (
            tc.tile_pool(name="wpool", bufs=2) as wpool,
            tc.tile_pool(name="io", bufs=3) as io,
            tc.tile_pool(name="hpool", bufs=8) as hpool,
            tc.tile_pool(name="ypool", bufs=8) as ypool,
            tc.tile_pool(name="psh", bufs=4, space="PSUM") as psh,
            tc.tile_pool(name="psy", bufs=4, space="PSUM") as psy,
        ):
            col0 = 0
            for s in range(nslot):
                cap = caps[s]
                widths = []
                off = 0
                while off < cap:
                    widths.append(min(BLK, cap - off))
                    off += widths[-1]

                # Emit the first block's x/cw loads BEFORE the weight loads:
                # everything shares the sync HWDGE queue (FIFO), and the
                # first matmul only needs x + w1 group 0.  All DMA triggers
                # stay off the scalar engine — trigger instructions cost
                # ~700ns each and block its in-order stream (delaying the
                # gelu table load and first gelu by several us).
                pre = {}
                w0 = widths[0]
                xt0 = io.tile([128, KD, w0], dt.bfloat16,
                              name=f"xt_{s}_0", tag="xt")
                nc.sync.dma_start(xt0[:], xg_d[:, :, col0:col0 + w0])
                cwt0 = io.tile([128, w0], dt.float32,
                               name=f"cw_{s}_0", tag="cw")
                nc.sync.dma_start(cwt0[:], cw_d[:, col0:col0 + w0])
                pre[0] = (xt0, cwt0)

                # Weight groups in consumption order: GEMM1 chunk c reads w1
                # group c, then GEMM2 chunk c reads w2 group c.
                w1ts, w2ts = [], []
                for g in range(F // BLK):
                    w1tg = wpool.tile([128, KD, BLK], dt.bfloat16,
                                      name=f"w1t{s}_{g}", tag=f"w1_{g}")
                    nc.sync.dma_start(w1tg[:], w1_d[s, g])
                    w1ts.append(w1tg)
                    w2tg = wpool.tile([128, BLK // 128, D], dt.bfloat16,
                                      name=f"w2t{s}_{g}", tag=f"w2_{g}")
                    nc.sync.dma_start(w2tg[:], w2_d[s, g])
                    w2ts.append(w2tg)
                if use_b1:
                    b1t = wpool.tile([128, KF], dt.float32,
                                     name=f"b1t{s}", tag="b1")
                    nc.sync.dma_start(b1t[:], b1_d[s])
                if use_b2:
                    b2t = wpool.tile([128, MD], dt.float32,
                                     name=f"b2t{s}", tag="b2")
                    nc.sync.dma_start(b2t[:], b2_d[s])

                off = 0
                for bi, w in enumerate(widths):
                    c0 = col0 + off
                    if bi in pre:
                        xt, cwt = pre[bi]
                    else:
                        xt = io.tile([128, KD, w], dt.bfloat16,
                                     name=f"xt_{s}_{off}", tag="xt")
                        nc.sync.dma_start(xt[:], xg_d[:, :, c0:c0 + w])
                        cwt = io.tile([128, w], dt.float32,
                                      name=f"cw_{s}_{off}", tag="cw")
                        nc.sync.dma_start(cwt[:], cw_d[:, c0:c0 + w])

                    yts = [psy.tile([128, w], dt.float32,
                                    name=f"psy_{s}_{off}_{m}", tag="psy")
                           for m in range(MD)]
                    n_fc = F // BLK           # 4 chunks of 512 hidden dims
                    fpc = BLK // 128          # 4 f-tiles per chunk
                    for c in range(n_fc):
                        hts = []
                        for f in range(fpc):
                            kk = c * fpc + f
                            ph = psh.tile([128, w], dt.float32,
                                          name=f"psh_{s}_{off}_{kk}", tag="psh")
                            for k in range(KD):
                                nc.tensor.matmul(
                                    ph[:],
                                    w1ts[c][:, k, f * 128:(f + 1) * 128],
                                    xt[:, k, :],
                                    start=(k == 0), stop=(k == KD - 1))
                            ht = hpool.tile([128, w], dt.bfloat16,
                                            name=f"ht_{s}_{off}_{kk}", tag="ht")
                            if use_b1:
                                nc.scalar.activation(ht[:], ph[:], gelu,
                                                     bias=b1t[:, kk:kk + 1])
                            else:
                                nc.scalar.activation(ht[:], ph[:], gelu)
                            hts.append(ht)
                        for m in range(MD):
                            for k in range(fpc):
                                nc.tensor.matmul(
                                    yts[m][:],
                                    w2ts[c][:, k, m * 128:(m + 1) * 128],
                                    hts[k][:],
                                    start=(c == 0 and k == 0),
                                    stop=(c == n_fc - 1 and k == fpc - 1))
                    for m in range(MD):
                        src = yts[m]
                        if use_b2:
                            yb = ypool.tile([128, w], dt.float32,
                                            name=f"yb_{s}_{off}_{m}", tag="yb")
                            nc.scalar.activation(yb[:], yts[m][:], ident,
                                                 bias=b2t[:, m:m + 1])
                            src = yb
                        yf = ypool.tile([128, w], dt.float32,
                                        name=f"yf_{s}_{off}_{m}", tag="yf")
                        nc.vector.tensor_mul(yf[:], src[:], cwt[:])
                        # Outputs on the gpsimd queue: keeps stores from
                        # delaying the next block's x/cw loads.
                        nc.gpsimd.dma_start(
                            yg_d[m * 128:(m + 1) * 128, c0:c0 + w], yf[:])
                    off += w
                col0 += cap
    nc.compile()
    _NC_CACHE[key] = nc
    return nc


def kernel(x, gate_w, w1, b1, w2, b2):
    global LAST_RESULT
    x = np.asarray(x, dtype=F32)
    gate_w = np.asarray(gate_w, dtype=F32)
    w1 = np.asarray(w1, dtype=F32)
    b1 = np.asarray(b1, dtype=F32)
    w2 = np.asarray(w2, dtype=F32)
    b2 = np.asarray(b2, dtype=F32)

    b, s, d = x.shape
    T = b * s
    xf = x.reshape(T, d)

    sel, topw = _gate(xf, gate_w)

    # Per-expert token lists: (expert, token idx, which-of-top2, weight)
    items = []
    for e in range(E):
        t0 = np.nonzero(sel[:, 0] == e)[0]
        t1 = np.nonzero(sel[:, 1] == e)[0]
        toks = np.concatenate([t0, t1])
        if toks.size == 0:
            continue
        ks = np.concatenate([np.zeros_like(t0), np.ones_like(t1)])
        ws = np.concatenate([topw[t0, 0], topw[t1, 1]]).astype(F32)
        items.append([e, toks, ks, ws])

    bins, caps = _plan(items)
    nslot = len(caps)
    ntok = sum(caps)
    col_of = np.concatenate([[0], np.cumsum(caps)])

    use_b1 = bool(np.any(b1))
    use_b2 = bool(np.any(b2))

    # Pack per-core device inputs.
    pos = np.zeros((T, TOPK), dtype=np.int64)
    in_maps = []
    for c in range(N_CORES):
        xg = np.zeros((128, KD, ntok), dtype=BF16)
        cw = np.zeros(ntok, dtype=F32)
        ng = F // BLK
        w1h = np.zeros((nslot, ng, 128, KD, BLK), dtype=BF16)
        w2h = np.zeros((nslot, ng, 128, BLK // 128, D), dtype=BF16)
        b1h = np.zeros((nslot, 128, KF), dtype=F32)
        b2h = np.zeros((nslot, 128, MD), dtype=F32)
        for j, (e, toks, ks, ws) in enumerate(bins[c]):
            o = int(col_of[j])
            n = len(toks)
            xg[:, :, o:o + n] = xf[toks].reshape(n, KD, 128).transpose(2, 1, 0)
            cw[o:o + n] = ws
            pos[toks, ks] = c * ntok + o + np.arange(n)
            for g in range(ng):
                w1h[j, g] = (w1[e][:, g * BLK:(g + 1) * BLK]
                             .reshape(KD, 128, BLK).transpose(1, 0, 2))
                w2h[j, g] = (w2[e][g * BLK:(g + 1) * BLK, :]
                             .reshape(BLK // 128, 128, D).transpose(1, 0, 2))
            b1h[j] = b1[e].reshape(KF, 128).T
            b2h[j] = b2[e].reshape(MD, 128).T
        m = {
            "xg": xg,
            "w1g": w1h,
            "w2g": w2h,
            "cwg": np.ascontiguousarray(
                np.broadcast_to(cw[None, :], (128, ntok))),
        }
        if use_b1:
            m["b1g"] = b1h
        if use_b2:
            m["b2g"] = b2h
        in_maps.append(m)

    nc = _build(nslot, caps, use_b1, use_b2)
    res = run_bass_kernel_spmd(nc, in_maps, core_ids=list(range(N_CORES)),
                               trace=TRACE, **TRACE_KW)
    LAST_RESULT = res

    yall = np.concatenate([res.results[c]["ygT"] for c in range(N_CORES)],
                          axis=1)                      # [D, N_CORES*ntok]
    out_t = yall[:, pos[:, 0]] + yall[:, pos[:, 1]]    # [D, T]
    out = np.ascontiguousarray(out_t.T).reshape(b, s, d).astype(F32)
    return out, np.float32(0.0)


# revision 12
# speedup vs baseline: 1.0430x; 1.0144x over previous
"""MoE layer (E=16 experts, top-2 routing) on 8 Trainium2 NeuronCores.

Strategy (expert-parallel, all-to-all dispatch done host-side):
  1. Host computes the gate (logits -> softmax -> top-2 -> renormalized
     combine weights) in fp32 numpy, exactly mirroring the reference ops.
  2. Tokens are gathered per expert; experts are bin-packed onto the 8
     cores (for the default load distribution this is exactly one
     5-block expert + one 4-block expert per core, no expert splitting).
  3. Each core runs the two expert FFNs on its gathered tokens with a
     Bass/Tile kernel: all activations live transposed ([feature, token])
     so both GEMMs keep weights stationary in the PE array and no
     on-chip transposes are needed.  bf16 inputs, fp32 PSUM accumulation,
     erf-Gelu on the scalar engine, combine-weight scaling on the vector
     engine.
  4. Host scatter-adds the two weighted contributions per token back to
     the [B, S, D] output.

Self-contained: hardcodes the problem shapes; no sibling imports.
"""

import numpy as np
import ml_dtypes

import concourse.bass as bass
import concourse.bacc as bacc
import concourse.mybir as mybir
import concourse.tile as tile
from concourse.bass_utils import run_bass_kernel_spmd

BF16 = ml_dtypes.bfloat16
F32 = np.float32

D = 512          # model dim
F = 2048         # FFN hidden dim
E = 16           # experts
TOPK = 2
N_CORES = 8
BLK = 512        # token block streamed per matmul (moving operand width)
KD = D // 128    # 4  contraction k-tiles for GEMM1
KF = F // 128    # 16 contraction k-tiles for GEMM2
MD = D // 128    # 4  output m-tiles for GEMM2

# Set by test harness only (profiling).  The graded path leaves these alone.
TRACE = False
TRACE_KW = {}
LAST_RESULT = None


def _gate(xf, gate_w):
    """fp32 gate identical to the reference: softmax -> top-2 -> renorm."""
    logits = xf @ gate_w                       # [T, E] fp32
    m = logits.max(axis=1, keepdims=True)
    ex = np.exp(logits - m)
    rw = ex / ex.sum(axis=1, keepdims=True)
    order = np.argsort(-rw, axis=1, kind="stable")
    sel = order[:, :TOPK]
    topw = np.take_along_axis(rw, sel, axis=1)
    topw = topw / topw.sum(axis=1, keepdims=True)
    return sel, topw.astype(F32)


def _plan(items):
    """Bin-pack expert workloads onto N_CORES cores.

    items: list of [expert, tokens, ks, ws] with len(tokens) > 0.
    Returns (bins, caps): bins[c] = list of items (sorted desc by size),
    caps[j] = token capacity (multiple of 128) of slot j, shared by all
    cores so every core compiles to the identical program.
    """
    nblk = lambda n: -(-n // BLK)
    btot = sum(nblk(len(it[1])) for it in items)
    bcap = max(1, -(-btot // N_CORES))
    while True:
        work = []
        for e, toks, ks, ws in items:
            o = 0
            while len(toks) - o > bcap * BLK:
                work.append([e, toks[o:o + bcap * BLK], ks[o:o + bcap * BLK],
                             ws[o:o + bcap * BLK]])
                o += bcap * BLK
            work.append([e, toks[o:], ks[o:], ws[o:]])
        work.sort(key=lambda it: -len(it[1]))
        bins = [[] for _ in range(N_CORES)]
        used = [0] * N_CORES
        ok = True
        for it in work:
            b = nblk(len(it[1]))
            for c in sorted(range(N_CORES), key=lambda c: used[c]):
                if used[c] + b <= bcap:
                    bins[c].append(it)
                    used[c] += b
                    break
            else:
                ok = False
                break
        if ok:
            break
        bcap += 1
    for b in bins:
        b.sort(key=lambda it: -len(it[1]))
    nslot = max(len(b) for b in bins)
    caps = []
    for j in range(nslot):
        cap = max((len(b[j][1]) if j < len(b) else 0) for b in bins)
        caps.append(-(-cap // 128) * 128)
    return bins, caps


_NC_CACHE = {}


def _build(nslot, caps, use_b1, use_b2):
    """Build the per-core Bass program (identical across cores)."""
    key = (nslot, tuple(caps), use_b1, use_b2)
    if key in _NC_CACHE:
        return _NC_CACHE[key]

    ntok = sum(caps)
    nc = bacc.Bacc("TRN2", target_bir_lowering=False, debug=False,
                   num_devices=N_CORES)
    dt = mybir.dt
    xg_d = nc.dram_tensor("xg", [128, KD, ntok], dt.bfloat16,
                          kind="ExternalInput").ap()
    # Weights grouped by consumption order, fully contiguous per partition
    # (4KB DMA lines): w1 group g = F-cols [g*512,(g+1)*512), w2 group c =
    # F-rows [c*512,(c+1)*512).
    w1_d = nc.dram_tensor("w1g", [nslot, F // BLK, 128, KD, BLK], dt.bfloat16,
                          kind="ExternalInput").ap()
    w2_d = nc.dram_tensor("w2g", [nslot, F // BLK, 128, BLK // 128, D],
                          dt.bfloat16, kind="ExternalInput").ap()
    cw_d = nc.dram_tensor("cwg", [128, ntok], dt.float32,
                          kind="ExternalInput").ap()
    b1_d = (nc.dram_tensor("b1g", [nslot, 128, KF], dt.float32,
                           kind="ExternalInput").ap() if use_b1 else None)
    b2_d = (nc.dram_tensor("b2g", [nslot, 128, MD], dt.float32,
                           kind="ExternalInput").ap() if use_b2 else None)
    yg_d = nc.dram_tensor("ygT", [D, ntok], dt.float32,
                          kind="ExternalOutput").ap()

    gelu = mybir.ActivationFunctionType.Gelu
    ident = mybir.ActivationFunctionType.Identity

    with tile.TileContext(nc) as tc:
        with (
            tc.tile_pool(name="wpool", bufs=2) as wpool,
            tc.tile_pool(name="io", bufs=3) as io,
            tc.tile_pool(name="hpool", bufs=8) as hpool,
            tc.tile_pool(name="ypool", bufs=8) as ypool,
            tc.tile_pool(name="psh", bufs=4, space="PSUM") as psh,
            tc.tile_pool(name="psy", bufs=4, space="PSUM") as psy,
        ):
            col0 = 0
            for s in range(nslot):
                cap = caps[s]
                widths = []
                off = 0
                while off < cap:
                    widths.append(min(BLK, cap - off))
                    off += widths[-1]
                # Remainder block first: the head-of-kernel x transfer is
                # smaller, so the first matmul starts sooner.
                widths.sort()

                # Emit the first block's x load BEFORE the weight loads:
                # everything shares the sync HWDGE queue (FIFO), and the
                # first matmul only needs x + w1 group 0.  All DMA triggers
                # stay off the scalar engine — trigger instructions cost
                # ~700ns each and block its in-order stream (delaying the
                # gelu table load and first gelu by several us).  cw is only
                # needed at the end of the block, so it queues after the
                # weights.
                pre = {}
                w0 = widths[0]
                xt0 = io.tile([128, KD, w0], dt.bfloat16,
                              name=f"xt_{s}_0", tag="xt")
                nc.sync.dma_start(xt0[:], xg_d[:, :, col0:col0 + w0])
                pre[0] = xt0

                # Weight groups in consumption order: GEMM1 chunk c reads w1
                # group c, then GEMM2 chunk c reads w2 group c.
                w1ts, w2ts = [], []
                for g in range(F // BLK):
                    w1tg = wpool.tile([128, KD, BLK], dt.bfloat16,
                                      name=f"w1t{s}_{g}", tag=f"w1_{g}")
                    nc.sync.dma_start(w1tg[:], w1_d[s, g])
                    w1ts.append(w1tg)
                    w2tg = wpool.tile([128, BLK // 128, D], dt.bfloat16,
                                      name=f"w2t{s}_{g}", tag=f"w2_{g}")
                    nc.sync.dma_start(w2tg[:], w2_d[s, g])
                    w2ts.append(w2tg)
                if use_b1:
                    b1t = wpool.tile([128, KF], dt.float32,
                                     name=f"b1t{s}", tag="b1")
                    nc.sync.dma_start(b1t[:], b1_d[s])
                if use_b2:
                    b2t = wpool.tile([128, MD], dt.float32,
                                     name=f"b2t{s}", tag="b2")
                    nc.sync.dma_start(b2t[:], b2_d[s])

                off = 0
                for bi, w in enumerate(widths):
                    c0 = col0 + off
                    if bi in pre:
                        xt = pre[bi]
                    else:
                        xt = io.tile([128, KD, w], dt.bfloat16,
                                     name=f"xt_{s}_{off}", tag="xt")
                        nc.sync.dma_start(xt[:], xg_d[:, :, c0:c0 + w])
                    cwt = io.tile([128, w], dt.float32,
                                  name=f"cw_{s}_{off}", tag="cw")
                    nc.sync.dma_start(cwt[:], cw_d[:, c0:c0 + w])

                    yts = [psy.tile([128, w], dt.float32,
                                    name=f"psy_{s}_{off}_{m}", tag="psy")
                           for m in range(MD)]
                    n_fc = F // BLK           # 4 chunks of 512 hidden dims
                    fpc = BLK // 128          # 4 f-tiles per chunk
                    for c in range(n_fc):
                        hts = []
                        for f in range(fpc):
                            kk = c * fpc + f
                            ph = psh.tile([128, w], dt.float32,
                                          name=f"psh_{s}_{off}_{kk}", tag="psh")
                            for k in range(KD):
                                nc.tensor.matmul(
                                    ph[:],
                                    w1ts[c][:, k, f * 128:(f + 1) * 128],
                                    xt[:, k, :],
                                    start=(k == 0), stop=(k == KD - 1))
                            ht = hpool.tile([128, w], dt.bfloat16,
                                            name=f"ht_{s}_{off}_{kk}", tag="ht")
                            if use_b1:
                                nc.scalar.activation(ht[:], ph[:], gelu,
                                                     bias=b1t[:, kk:kk + 1])
                            else:
                                nc.scalar.activation(ht[:], ph[:], gelu)
                            hts.append(ht)
                        for m in range(MD):
                            for k in range(fpc):
                                nc.tensor.matmul(
                                    yts[m][:],
                                    w2ts[c][:, k, m * 128:(m + 1) * 128],
                                    hts[k][:],
                                    start=(c == 0 and k == 0),
                                    stop=(c == n_fc - 1 and k == fpc - 1))
                    for m in range(MD):
                        src = yts[m]
                        if use_b2:
                            yb = ypool.tile([128, w], dt.float32,
                                            name=f"yb_{s}_{off}_{m}", tag="yb")
                            nc.scalar.activation(yb[:], yts[m][:], ident,
                                                 bias=b2t[:, m:m + 1])
                            src = yb
                        yf = ypool.tile([128, w], dt.float32,
                                        name=f"yf_{s}_{off}_{m}", tag="yf")
                        nc.vector.tensor_mul(yf[:], src[:], cwt[:])
                        # Stores share the sync queue; they are emitted after
                        # the next block's loads in FIFO order, so prefetch
                        # is unharmed, and HWDGE drains the tail far faster
                        # than the gpsimd SWDGE queue.
                        nc.sync.dma_start(
                            yg_d[m * 128:(m + 1) * 128, c0:c0 + w], yf[:])
                    off += w
                col0 += cap
    nc.compile()
    _NC_CACHE[key] = nc
    return nc


def kernel(x, gate_w, w1, b1, w2, b2):
    global LAST_RESULT
    x = np.asarray(x, dtype=F32)
    gate_w = np.asarray(gate_w, dtype=F32)
    w1 = np.asarray(w1, dtype=F32)
    b1 = np.asarray(b1, dtype=F32)
    w2 = np.asarray(w2, dtype=F32)
    b2 = np.asarray(b2, dtype=F32)

    b, s, d = x.shape
    T = b * s
    xf = x.reshape(T, d)

    sel, topw = _gate(xf, gate_w)

    # Per-expert token lists: (expert, token idx, which-of-top2, weight)
    items = []
    for e in range(E):
        t0 = np.nonzero(sel[:, 0] == e)[0]
        t1 = np.nonzero(sel[:, 1] == e)[0]
        toks = np.concatenate([t0, t1])
        if toks.size == 0:
            continue
        ks = np.concatenate([np.zeros_like(t0), np.ones_like(t1)])
        ws = np.concatenate([topw[t0, 0], topw[t1, 1]]).astype(F32)
        items.append([e, toks, ks, ws])

    bins, caps = _plan(items)
    nslot = len(caps)
    ntok = sum(caps)
    col_of = np.concatenate([[0], np.cumsum(caps)])

    use_b1 = bool(np.any(b1))
    use_b2 = bool(np.any(b2))

    # Pack per-core device inputs.
    pos = np.zeros((T, TOPK), dtype=np.int64)
    in_maps = []
    for c in range(N_CORES):
        xg = np.zeros((128, KD, ntok), dtype=BF16)
        cw = np.zeros(ntok, dtype=F32)
        ng = F // BLK
        w1h = np.zeros((nslot, ng, 128, KD, BLK), dtype=BF16)
        w2h = np.zeros((nslot, ng, 128, BLK // 128, D), dtype=BF16)
        b1h = np.zeros((nslot, 128, KF), dtype=F32)
        b2h = np.zeros((nslot, 128, MD), dtype=F32)
        for j, (e, toks, ks, ws) in enumerate(bins[c]):
            o = int(col_of[j])
            n = len(toks)
            xg[:, :, o:o + n] = xf[toks].reshape(n, KD, 128).transpose(2, 1, 0)
            cw[o:o + n] = ws
            pos[toks, ks] = c * ntok + o + np.arange(n)
            for g in range(ng):
                w1h[j, g] = (w1[e][:, g * BLK:(g + 1) * BLK]
                             .reshape(KD, 128, BLK).transpose(1, 0, 2))
                w2h[j, g] = (w2[e][g * BLK:(g + 1) * BLK, :]
                             .reshape(BLK // 128, 128, D).transpose(1, 0, 2))
            b1h[j] = b1[e].reshape(KF, 128).T
            b2h[j] = b2[e].reshape(MD, 128).T
        m = {
            "xg": xg,
            "w1g": w1h,
            "w2g": w2h,
            "cwg": np.ascontiguousarray(
                np.broadcast_to(cw[None, :], (128, ntok))),
        }
        if use_b1:
            m["b1g"] = b1h
        if use_b2:
            m["b2g"] = b2h
        in_maps.append(m)

    nc = _build(nslot, caps, use_b1, use_b2)
    res = run_bass_kernel_spmd(nc, in_maps, core_ids=list(range(N_CORES)),
                               trace=TRACE, **TRACE_KW)
    LAST_RESULT = res

    yall = np.concatenate([res.results[c]["ygT"] for c in range(N_CORES)],
                          axis=1)                      # [D, N_CORES*ntok]
    out_t = yall[:, pos[:, 0]] + yall[:, pos[:, 1]]    # [D, T]
    out = np.ascontiguousarray(out_t.T).reshape(b, s, d).astype(F32)
    return out, np.float32(0.0)
